# revision 37
# baseline (speedup 1.0000x reference)
"""GQA forward (B=2,T=2048,D=1024,H=16,KV=4,HD=64) on 8 TRN2 NeuronCores.

Sharding: core c -> (batch b=c//4, kv-group g=c%4). Each core computes the
4 query heads of its kv group against its batch, plus the partial output
projection for its 256 columns of the concat-head activation; the host sums
the 4 per-group partials of each batch (row-parallel out_proj unshard).

v3 pipeline: attention runs as 8 units (head-pair hp x T-quarter tq), each
16 slices of [128 s, {head 2hp | head 2hp+1} x 512 q] scores. Score PSUM is
a 3-deep ring ([128,1024] f32 = 2 banks each) and the per-unit PV
accumulators are [65, 512] (1 bank each), so QK can run 2 slices ahead of
the softmax: emission order per slice is exp(i), QK(i+2), PV(i), which
keeps the exp->QK->exp buffer chain off the critical path. exp is split
9:7 between ACT (table exp, scale=1/8) and DVE (Schraudolph bit-trick:
one tensor_scalar mult+add fp32->int16 whose bits are bf16(exp(x/8)+-3%),
consumed by PV through a bf16 bitcast AP; num/den share P so the wiggle
mostly cancels).

Projections: k and v are computed in one col-tiled pass (stationary
[kwT|vwT], two concurrent matmuls into PSUM rows 0-63/64-127). v is moved
to (s, d) layout with the DMA xbar transpose (16 x (64,128)->(128,64)),
not matmuls. RoPE runs at 1024-wide DVE ops in (d, t) layout: pair-swap
via stream_shuffle, then q*cosF + swap(q)*sinF with sign folded into sinF
on the host. No max-subtraction in softmax: |scores/8| < ~4 here.
"""

import os
import sys

for _p in ("/opt/trn_rl_repo",):
    if _p not in sys.path:
        sys.path.insert(0, _p)

import numpy as np

B, T, D = 2, 2048, 1024
H, KV, HD = 16, 4, 64
REP = H // KV          # 4 query heads per core
GH = REP * HD          # 256 q columns per core
P = 128
SC = T // P            # s-chunks (contraction tiles over sequence)
KC = D // P            # k-chunks over the model dim

SWAP_MASK = [i ^ 1 for i in range(32)]  # pair-swap within each 32-partition quadrant

LOG2E = 1.4426950408889634
# Schraudolph constants for bf16-bits-in-int16: round((x/8)*a' + b') == bf16 bits
# of exp(x/8) with max rel err ~3%; attention scale 1/8 folded into the slope.
SCH_A = 128.0 * LOG2E / 8.0
SCH_B = 128.0 * (127.0 - 0.04303)

_MODULE_CACHE = {}
LAST_RESULT = None  # test.py reads exec_time_ns / trace path from here


def _build():
    import concourse.tile as tile
    from concourse import mybir
    from concourse.bacc import Bacc

    bf16 = mybir.dt.bfloat16
    f32 = mybir.dt.float32
    i16 = mybir.dt.int16
    AF = mybir.ActivationFunctionType
    ALU = mybir.AluOpType

    nc = Bacc(trn_type="TRN2")
    xT_h = nc.dram_tensor("xT", (D, T), bf16, kind="ExternalInput")
    qwT_h = nc.dram_tensor("qwT", (D, GH), bf16, kind="ExternalInput")
    kvwT_h = nc.dram_tensor("kvwT", (D, P), bf16, kind="ExternalInput")
    owT_h = nc.dram_tensor("owT", (GH, D), bf16, kind="ExternalInput")
    cos_h = nc.dram_tensor("cosF", (P, T), f32, kind="ExternalInput")
    sin_h = nc.dram_tensor("sinF", (P, T), f32, kind="ExternalInput")
    out_h = nc.dram_tensor("outT", (D, T), bf16, kind="ExternalOutput")
    debug = os.environ.get("KERNEL_DEBUG", "0") == "1"
    if debug:
        dbg_kdup_h = nc.dram_tensor("dbg_kdup", (P, T), bf16, kind="ExternalOutput")
        dbg_v_h = nc.dram_tensor("dbg_v", (P, 4 * 288), bf16,
                                 kind="ExternalOutput")
        dbg_qro_h = nc.dram_tensor("dbg_qro", (P, 2 * T), bf16,
                                   kind="ExternalOutput")
        dbg_ot_h = nc.dram_tensor("dbg_ot", (P, 2 * T), bf16, kind="ExternalOutput")

    xTr = xT_h[:, :].rearrange("(c p) t -> p c t", p=P)
    qwTr = qwT_h[:, :].rearrange("(c p) m -> p c m", p=P)
    kvwTr = kvwT_h[:, :].rearrange("(c p) m -> p c m", p=P)
    owTr = owT_h[:, :].rearrange("(c p) n -> p c n", p=P)
    outr = out_h[:, :].rearrange("(c p) t -> p c t", p=P)

    with tile.TileContext(nc) as tc:
        with (
            tc.tile_pool(name="consts", bufs=1) as consts,
            tc.tile_pool(name="rope", bufs=2) as rope,
            tc.tile_pool(name="pexp", bufs=3) as pexp,
            tc.tile_pool(name="psch", bufs=3) as psch,
            tc.tile_pool(name="norm", bufs=2) as norm,
            tc.tile_pool(name="outs", bufs=2) as outs,
            tc.tile_pool(name="ps_sc", bufs=3, space="PSUM") as ps_sc,
            tc.tile_pool(name="ps_ot", bufs=1, space="PSUM") as ps_ot,
        ):
            # ---- loads: one tile per xT k-chunk so matmuls start per-chunk ----
            x_sb, qw_sb = [], []
            for c in range(KC):
                qc = consts.tile([P, GH], bf16, tag=f"qw{c}", name=f"qw{c}")
                nc.sync.dma_start(out=qc, in_=qwTr[:, c, :])
                qw_sb.append(qc)
                xc = consts.tile([P, T], bf16, tag=f"x{c}", name=f"x{c}")
                nc.sync.dma_start(out=xc, in_=xTr[:, c, :])
                x_sb.append(xc)
            # touch the exp table set at t=0 so the ~2.7us ACT_TABLE_LOAD
            # hides under the projection phase instead of the first softmax
            warm = consts.tile([1, 8], f32, name="warm")
            nc.vector.memset(warm[:, :], 0.0)
            nc.scalar.activation(warm[:, :], warm[:, :], AF.Exp)
            kvw_sb = consts.tile([P, KC, P], bf16)
            nc.sync.dma_start(out=kvw_sb, in_=kvwTr)
            cos_sb = consts.tile([P, T], f32)
            nc.sync.dma_start(out=cos_sb, in_=cos_h[:, :])
            sin_sb = consts.tile([P, T], f32)
            nc.sync.dma_start(out=sin_sb, in_=sin_h[:, :])
            owT_sb = consts.tile([P, 2, D], bf16)
            nc.sync.dma_start(out=owT_sb, in_=owTr)

            # split tiles so attention slices unblock as soon as their s-range
            # is projected (dependencies are tracked per tile)
            qro4 = [[consts.tile([P, 1024], bf16, tag=f"qro{m}{h}",
                                 name=f"qro{m}{h}") for h in range(2)]
                    for m in range(2)]
            kd2 = [consts.tile([P, 1024], bf16, tag=f"kd{h}", name=f"kd{h}")
                   for h in range(2)]
            # v chunk stride padded to 72 elems (144B, 16B-aligned) -- the DMA
            # xbar transpose needs an aligned destination offset
            v_sb4 = [consts.tile([P, 4, 72], bf16, tag=f"v{j}", name=f"v{j}")
                     for j in range(4)]
            for j in range(4):
                nc.vector.memset(v_sb4[j][:, :, HD : HD + 1], 1.0)
            ot_sb = consts.tile([P, 2, T], bf16)

            def rope_1024(ps, out_ap, tsl, p_sz):
                # out = ps*cos + swap(ps)*sin over a [p_sz, 1024] psum tile
                sw = rope.tile([P, 1024], f32, tag="sw")
                nc.vector.stream_shuffle(sw[:p_sz], ps, SWAP_MASK)
                t1 = rope.tile([P, 1024], f32, tag="t1")
                nc.vector.tensor_mul(t1[:p_sz], ps, cos_sb[:p_sz, tsl])
                nc.vector.tensor_mul(sw[:p_sz], sw[:p_sz], sin_sb[:p_sz, tsl])
                nc.vector.tensor_add(out_ap, t1[:p_sz], sw[:p_sz])

            # ---- q projection chunk m, one T-half: a [128,1024] psum tile ----
            def qproj_half(m, half):
                hsl = slice(half * 1024, (half + 1) * 1024)
                ps = ps_sc.tile([P, 1024], f32, tag="sc", name=f"qps{m}{half}")
                for t in range(2):
                    tsl = slice(half * 1024 + t * 512, half * 1024 + (t + 1) * 512)
                    psl = slice(t * 512, (t + 1) * 512)
                    for c in range(KC):
                        nc.tensor.matmul(
                            ps[:, psl],
                            lhsT=qw_sb[c][:, m * P : (m + 1) * P],
                            rhs=x_sb[c][:, tsl],
                            start=(c == 0),
                            stop=(c == KC - 1),
                        )
                rope_1024(ps[:, :], qro4[m][half][:, :], hsl, P)

            def qproj(m):
                qproj_half(m, 0)
                qproj_half(m, 1)

            qproj(0)

            # ---- k projection (rows 0-63 of the fused kv weights) ----
            for half in range(2):
                hsl = slice(half * 1024, (half + 1) * 1024)
                kps = ps_sc.tile([P, 1024], f32, tag="sc", name=f"kps{half}")
                for t in range(2):
                    tsl = slice(half * 1024 + t * 512, half * 1024 + (t + 1) * 512)
                    psl = slice(t * 512, (t + 1) * 512)
                    for c in range(KC):
                        nc.tensor.matmul(
                            kps[0:HD, psl],
                            lhsT=kvw_sb[:, c, 0:HD],
                            rhs=x_sb[c][:, tsl],
                            start=(c == 0), stop=(c == KC - 1),
                        )
                rope_1024(kps[0:HD, :], kd2[half][0:HD, :], hsl, HD)
                nc.vector.tensor_copy(kd2[half][HD:P, :], kd2[half][0:HD, :])

            # ---- v projection: (s, d) via per-s-chunk matmuls ----
            for half in range(2):
                vps = ps_sc.tile([P, 1024], f32, tag="sc", name=f"vps{half}")
                for s in range(half * 8, half * 8 + 8):
                    idx = s % 8
                    vsl = slice(idx * HD, (idx + 1) * HD)
                    for c in range(KC):
                        nc.tensor.matmul(
                            vps[:, vsl],
                            lhsT=x_sb[c][:, s * P : (s + 1) * P],
                            rhs=kvw_sb[:, c, HD:P],
                            start=(c == 0), stop=(c == KC - 1),
                        )
                for s in range(half * 8, half * 8 + 8):
                    idx = s % 8
                    nc.scalar.copy(v_sb4[s // 4][:, s % 4, 0:HD],
                                   vps[:, idx * HD : (idx + 1) * HD])


            # ---- attention: units (hp, tq); 16 slices (s) each ----
            # slice scores [128, 1024] = {head 2hp q-block tq | head 2hp+1 same}
            # Emission per slice i: exp(i), QK(i+2), PV(i) -- the PE streams two
            # slices ahead, and with 3 score buffers the exp engines decouple.
            scale = 1.0 / float(np.sqrt(HD))
            units = [(hp, tq) for hp in range(2) for tq in range(4)]
            slices = [(u, s) for u in range(8) for s in range(SC)]
            ot_tiles = {}

            def emit_qk(i):
                u, s = slices[i]
                hp, tq = units[u]
                kd = kd2[s // 8]
                ssl = slice((s % 8) * P, (s % 8 + 1) * P)
                qr = qro4[hp][tq // 2]
                src = slice((tq % 2) * 512, (tq % 2) * 512 + 512)
                cur = ps_sc.tile([P, 1024], f32, tag="sc")
                nc.tensor.matmul(cur[:, 0:512], lhsT=kd[0:64, ssl],
                                 rhs=qr[0:64, src], start=True, stop=True)
                nc.tensor.matmul(cur[:, 512:1024], lhsT=kd[64:P, ssl],
                                 rhs=qr[64:P, src], start=True, stop=True)
                return cur

            def emit_norm(u, otA, otB):
                hp, tq = units[u]
                to = tq * 512
                # copy O^T+denom out of PSUM (frees ot tags for next unit);
                # otA via ACT, otB via DVE to split the copy cost
                for half, ot in ((0, otA), (1, otB)):
                    rows = slice(64 * half, 64 * half + 64)
                    of = norm.tile([HD + 1, 512], f32, tag=f"of{half}",
                                   name=f"of{half}")
                    nc.scalar.copy(of, ot[: HD + 1, :])
                    # custom-DVE recip and partition_broadcast both misbehave
                    # on base-partition-64 inputs; hop the denom row to
                    # partition 0 with a tiny sbuf->sbuf DMA first
                    dn = norm.tile([1, 512], f32, tag="dn")
                    nc.sync.dma_start(out=dn, in_=of[HD : HD + 1, :])
                    recip = norm.tile([1, 512], f32, tag="recip")
                    nc.vector.reciprocal_approx_fast(recip, dn)
                    rb = norm.tile([HD, 512], f32, tag="rb")
                    nc.gpsimd.partition_broadcast(rb, recip)
                    nc.vector.tensor_mul(
                        ot_sb[rows, hp, to : to + 512], of[0:HD, :], rb
                    )

            qk_bufs = {0: emit_qk(0), 1: emit_qk(1)}
            for i, (u, s) in enumerate(slices):
                # q chunk 1 (heads 2-3) is projected inside the hp0 units --
                # it is first needed at slice 64, so these small matmul lumps
                # hide in the attention stream instead of lengthening the head
                if i == 8:
                    qproj_half(1, 0)
                elif i == 24:
                    qproj_half(1, 1)
                cur = qk_bufs.pop(i)
                if s == 0:
                    ot_tiles[u] = (ps_ot.tile([HD + 1, 512], f32, tag="otA",
                                              name=f"uotA{u}"),
                                   ps_ot.tile([HD + 1, 512], f32, tag="otB",
                                              name=f"uotB{u}"))
                otA, otB = ot_tiles[u]
                # exp: 10 of every 16 slices on ACT, 6 on DVE, interleaved
                if (i * 10) % 16 < 10:
                    pt = pexp.tile([P, 1024], bf16, tag="p")
                    nc.scalar.activation(pt, cur, AF.Exp, scale=scale)
                    pA, pB = pt[:, 0:512], pt[:, 512:1024]
                else:
                    st = psch.tile([P, 1024], i16, tag="q")
                    nc.vector.tensor_scalar(
                        out=st[:, :], in0=cur[:, :],
                        scalar1=SCH_A, scalar2=SCH_B,
                        op0=ALU.mult, op1=ALU.add,
                    )
                    pA = st[:, 0:512].bitcast(bf16)
                    pB = st[:, 512:1024].bitcast(bf16)
                if i + 2 < len(slices):
                    qk_bufs[i + 2] = emit_qk(i + 2)
                vst = v_sb4[s // 4][:, s % 4, 0 : HD + 1]
                nc.tensor.matmul(otA[:, :], lhsT=vst, rhs=pA,
                                 start=(s == 0), stop=(s == SC - 1))
                nc.tensor.matmul(otB[:, :], lhsT=vst, rhs=pB,
                                 start=(s == 0), stop=(s == SC - 1))
                if s == SC - 1:
                    emit_norm(u, otA, otB)

            if debug:
                for h in range(2):
                    nc.sync.dma_start(out=dbg_kdup_h[:, h * 1024 : (h + 1) * 1024],
                                      in_=kd2[h])
                for j in range(4):
                    nc.sync.dma_start(
                        out=dbg_v_h[:, j * 288 : (j + 1) * 288],
                        in_=v_sb4[j][:, :, :].rearrange("p a b -> p (a b)"),
                    )
                for m in range(2):
                    for h in range(2):
                        nc.sync.dma_start(
                            out=dbg_qro_h[:, (m * 2 + h) * 1024 : (m * 2 + h + 1) * 1024],
                            in_=qro4[m][h],
                        )
                nc.sync.dma_start(
                    out=dbg_ot_h[:, :],
                    in_=ot_sb[:, :, :].rearrange("p a b -> p (a b)"),
                )

            # ---- output projection: ps_sc ring provides the psum tiles ----
            for oc in range(KC):
                osl = slice(oc * P, (oc + 1) * P)
                halves = [ps_sc.tile([P, 1024], f32, tag="sc", name=f"ops{oc}a"),
                          ps_sc.tile([P, 1024], f32, tag="sc", name=f"ops{oc}b")]
                for th in range(2):
                    for t in range(2):
                        psl = slice(t * 512, (t + 1) * 512)
                        tsl = slice(th * 1024 + t * 512, th * 1024 + (t + 1) * 512)
                        for c in range(2):
                            nc.tensor.matmul(
                                halves[th][:, psl],
                                lhsT=owT_sb[:, c, osl],
                                rhs=ot_sb[:, c, tsl],
                                start=(c == 0),
                                stop=(c == 1),
                            )
                o_sb = outs.tile([P, T], bf16, tag="o")
                for th in range(2):
                    dst = o_sb[:, th * 1024 : (th + 1) * 1024]
                    if (oc + th) % 2 == 0:
                        nc.vector.tensor_copy(dst, halves[th])
                    else:
                        nc.scalar.copy(dst, halves[th])
                nc.sync.dma_start(out=outr[:, oc, :], in_=o_sb)

    nc.finalize()
    return nc


def _get_module():
    if "nc" not in _MODULE_CACHE:
        _MODULE_CACHE["nc"] = _build()
    return _MODULE_CACHE["nc"]


def _host_freqs(freqs_cos, freqs_sin):
    cos = np.asarray(freqs_cos, dtype=np.float32)  # (T, 32)
    sin = np.asarray(freqs_sin, dtype=np.float32)
    c64 = np.repeat(cos, 2, axis=1)                # (T, 64): col d -> cos[t, d//2]
    s64 = np.empty((T, HD), dtype=np.float32)
    s64[:, 0::2] = -sin
    s64[:, 1::2] = sin
    cosF = np.ascontiguousarray(np.concatenate([c64, c64], axis=1).T)  # (128, T)
    sinF = np.ascontiguousarray(np.concatenate([s64, s64], axis=1).T)
    return cosF, sinF


def kernel(x, q_w, kv_w, out_w, freqs_cos, freqs_sin):
    global LAST_RESULT
    import ml_dtypes
    from concourse.bass_utils import run_bass_kernel_spmd

    bf = ml_dtypes.bfloat16
    x = np.asarray(x, dtype=np.float32)
    q_w = np.asarray(q_w, dtype=np.float32)
    kv_w = np.asarray(kv_w, dtype=np.float32)
    out_w = np.asarray(out_w, dtype=np.float32)
    cosF, sinF = _host_freqs(freqs_cos, freqs_sin)

    xT = [np.ascontiguousarray(x[b].T).astype(bf) for b in range(B)]
    in_maps = []
    for core in range(8):
        b, g = core // KV, core % KV
        kvwT = np.concatenate(
            [kv_w[g * HD : (g + 1) * HD, :].T,
             kv_w[(KV + g) * HD : (KV + g + 1) * HD, :].T], axis=1
        )  # (D, 128): [kT | vT]
        in_maps.append(
            dict(
                xT=xT[b],
                qwT=np.ascontiguousarray(q_w[g * GH : (g + 1) * GH, :].T).astype(bf),
                kvwT=np.ascontiguousarray(kvwT).astype(bf),
                owT=np.ascontiguousarray(out_w[:, g * GH : (g + 1) * GH].T).astype(bf),
                cosF=cosF,
                sinF=sinF,
            )
        )

    nc = _get_module()
    trace = os.environ.get("KERNEL_TRACE", "0") == "1"
    res = run_bass_kernel_spmd(nc, in_maps, core_ids=list(range(8)), trace=trace)
    LAST_RESULT = res

    out = np.zeros((B, T, D), dtype=np.float32)
    for core in range(8):
        b = core // KV
        out[b] += res.results[core]["outT"].T.astype(np.float32)
    return out


# revision 39
# speedup vs baseline: 1.0130x; 1.0130x over previous
"""GQA forward (B=2,T=2048,D=1024,H=16,KV=4,HD=64) on 8 TRN2 NeuronCores.

Sharding: core c -> (batch b=c//4, kv-group g=c%4). Each core computes the
4 query heads of its kv group against its batch, plus the partial output
projection for its 256 columns of the concat-head activation; the host sums
the 4 per-group partials of each batch (row-parallel out_proj unshard).

v3 pipeline: attention runs as 8 units (head-pair hp x T-quarter tq), each
16 slices of [128 s, {head 2hp | head 2hp+1} x 512 q] scores. Score PSUM is
a 3-deep ring ([128,1024] f32 = 2 banks each) and the per-unit PV
accumulators are [65, 512] (1 bank each), so QK can run 2 slices ahead of
the softmax: emission order per slice is exp(i), QK(i+2), PV(i), which
keeps the exp->QK->exp buffer chain off the critical path. exp is split
9:7 between ACT (table exp, scale=1/8) and DVE (Schraudolph bit-trick:
one tensor_scalar mult+add fp32->int16 whose bits are bf16(exp(x/8)+-3%),
consumed by PV through a bf16 bitcast AP; num/den share P so the wiggle
mostly cancels).

Projections: k and v are computed in one col-tiled pass (stationary
[kwT|vwT], two concurrent matmuls into PSUM rows 0-63/64-127). v is moved
to (s, d) layout with the DMA xbar transpose (16 x (64,128)->(128,64)),
not matmuls. RoPE runs at 1024-wide DVE ops in (d, t) layout: pair-swap
via stream_shuffle, then q*cosF + swap(q)*sinF with sign folded into sinF
on the host. No max-subtraction in softmax: |scores/8| < ~4 here.
"""

import os
import sys

for _p in ("/opt/trn_rl_repo",):
    if _p not in sys.path:
        sys.path.insert(0, _p)

import numpy as np

B, T, D = 2, 2048, 1024
H, KV, HD = 16, 4, 64
REP = H // KV          # 4 query heads per core
GH = REP * HD          # 256 q columns per core
P = 128
SC = T // P            # s-chunks (contraction tiles over sequence)
KC = D // P            # k-chunks over the model dim

SWAP_MASK = [i ^ 1 for i in range(32)]  # pair-swap within each 32-partition quadrant

LOG2E = 1.4426950408889634
# Schraudolph constants for bf16-bits-in-int16: round((x/8)*a' + b') == bf16 bits
# of exp(x/8) with max rel err ~3%; attention scale 1/8 folded into the slope.
SCH_A = 128.0 * LOG2E / 8.0
SCH_B = 128.0 * (127.0 - 0.04303)

_MODULE_CACHE = {}
LAST_RESULT = None  # test.py reads exec_time_ns / trace path from here


def _build():
    import concourse.tile as tile
    from concourse import mybir
    from concourse.bacc import Bacc

    bf16 = mybir.dt.bfloat16
    f32 = mybir.dt.float32
    i16 = mybir.dt.int16
    AF = mybir.ActivationFunctionType
    ALU = mybir.AluOpType

    nc = Bacc(trn_type="TRN2")
    xT_h = nc.dram_tensor("xT", (D, T), bf16, kind="ExternalInput")
    qwT_h = nc.dram_tensor("qwT", (D, GH), bf16, kind="ExternalInput")
    kvwT_h = nc.dram_tensor("kvwT", (D, P), bf16, kind="ExternalInput")
    owT_h = nc.dram_tensor("owT", (GH, D), bf16, kind="ExternalInput")
    cos_h = nc.dram_tensor("cosF", (P, T), f32, kind="ExternalInput")
    sin_h = nc.dram_tensor("sinF", (P, T), f32, kind="ExternalInput")
    out_h = nc.dram_tensor("outT", (D, T), bf16, kind="ExternalOutput")
    debug = os.environ.get("KERNEL_DEBUG", "0") == "1"
    if debug:
        dbg_kdup_h = nc.dram_tensor("dbg_kdup", (P, T), bf16, kind="ExternalOutput")
        dbg_v_h = nc.dram_tensor("dbg_v", (P, 4 * 288), bf16,
                                 kind="ExternalOutput")
        dbg_qro_h = nc.dram_tensor("dbg_qro", (P, 2 * T), bf16,
                                   kind="ExternalOutput")
        dbg_ot_h = nc.dram_tensor("dbg_ot", (P, 2 * T), bf16, kind="ExternalOutput")

    xTr = xT_h[:, :].rearrange("(c p) t -> p c t", p=P)
    qwTr = qwT_h[:, :].rearrange("(c p) m -> p c m", p=P)
    kvwTr = kvwT_h[:, :].rearrange("(c p) m -> p c m", p=P)
    owTr = owT_h[:, :].rearrange("(c p) n -> p c n", p=P)
    outr = out_h[:, :].rearrange("(c p) t -> p c t", p=P)

    with tile.TileContext(nc) as tc:
        with (
            tc.tile_pool(name="consts", bufs=1) as consts,
            tc.tile_pool(name="rope", bufs=2) as rope,
            tc.tile_pool(name="pexp", bufs=2) as pexp,
            tc.tile_pool(name="psch", bufs=2) as psch,
            tc.tile_pool(name="norm", bufs=2) as norm,
            tc.tile_pool(name="outs", bufs=2) as outs,
            tc.tile_pool(name="ps_sc", bufs=3, space="PSUM") as ps_sc,
            tc.tile_pool(name="ps_ot", bufs=1, space="PSUM") as ps_ot,
        ):
            # ---- loads: one tile per xT k-chunk so matmuls start per-chunk ----
            x_sb, qw_sb = [], []
            for c in range(KC):
                qc = consts.tile([P, GH], bf16, tag=f"qw{c}", name=f"qw{c}")
                nc.sync.dma_start(out=qc, in_=qwTr[:, c, :])
                qw_sb.append(qc)
                xc = consts.tile([P, T], bf16, tag=f"x{c}", name=f"x{c}")
                nc.sync.dma_start(out=xc, in_=xTr[:, c, :])
                x_sb.append(xc)
            kvw_sb = consts.tile([P, KC, P], bf16)
            nc.sync.dma_start(out=kvw_sb, in_=kvwTr)
            cos_sb = consts.tile([P, T], f32)
            nc.sync.dma_start(out=cos_sb, in_=cos_h[:, :])
            sin_sb = consts.tile([P, T], f32)
            nc.sync.dma_start(out=sin_sb, in_=sin_h[:, :])
            owT_sb = consts.tile([P, 2, D], bf16)
            nc.sync.dma_start(out=owT_sb, in_=owTr)

            # split tiles so attention slices unblock as soon as their s-range
            # is projected (dependencies are tracked per tile)
            qro4 = [[consts.tile([P, 1024], bf16, tag=f"qro{m}{h}",
                                 name=f"qro{m}{h}") for h in range(2)]
                    for m in range(2)]
            kd2 = [consts.tile([P, 1024], bf16, tag=f"kd{h}", name=f"kd{h}")
                   for h in range(2)]
            # v chunk stride padded to 72 elems (144B, 16B-aligned) -- the DMA
            # xbar transpose needs an aligned destination offset
            v_sb4 = [consts.tile([P, 4, 72], bf16, tag=f"v{j}", name=f"v{j}")
                     for j in range(4)]
            for j in range(4):
                nc.vector.memset(v_sb4[j][:, :, HD : HD + 1], 1.0)
            ot_sb = consts.tile([P, 2, T], bf16)

            def rope_1024(ps, out_ap, tsl, p_sz):
                # out = ps*cos + swap(ps)*sin over a [p_sz, 1024] psum tile
                sw = rope.tile([P, 1024], f32, tag="sw")
                nc.vector.stream_shuffle(sw[:p_sz], ps, SWAP_MASK)
                t1 = rope.tile([P, 1024], f32, tag="t1")
                nc.vector.tensor_mul(t1[:p_sz], ps, cos_sb[:p_sz, tsl])
                nc.vector.tensor_mul(sw[:p_sz], sw[:p_sz], sin_sb[:p_sz, tsl])
                nc.vector.tensor_add(out_ap, t1[:p_sz], sw[:p_sz])

            # ---- q projection chunk m, one T-half: a [128,1024] psum tile ----
            def qproj_half(m, half):
                hsl = slice(half * 1024, (half + 1) * 1024)
                ps = ps_sc.tile([P, 1024], f32, tag="sc", name=f"qps{m}{half}")
                for t in range(2):
                    tsl = slice(half * 1024 + t * 512, half * 1024 + (t + 1) * 512)
                    psl = slice(t * 512, (t + 1) * 512)
                    for c in range(KC):
                        nc.tensor.matmul(
                            ps[:, psl],
                            lhsT=qw_sb[c][:, m * P : (m + 1) * P],
                            rhs=x_sb[c][:, tsl],
                            start=(c == 0),
                            stop=(c == KC - 1),
                        )
                rope_1024(ps[:, :], qro4[m][half][:, :], hsl, P)

            def qproj(m):
                qproj_half(m, 0)
                qproj_half(m, 1)

            qproj(0)

            # ---- k projection (rows 0-63 of the fused kv weights) ----
            for half in range(2):
                hsl = slice(half * 1024, (half + 1) * 1024)
                kps = ps_sc.tile([P, 1024], f32, tag="sc", name=f"kps{half}")
                for t in range(2):
                    tsl = slice(half * 1024 + t * 512, half * 1024 + (t + 1) * 512)
                    psl = slice(t * 512, (t + 1) * 512)
                    for c in range(KC):
                        nc.tensor.matmul(
                            kps[0:HD, psl],
                            lhsT=kvw_sb[:, c, 0:HD],
                            rhs=x_sb[c][:, tsl],
                            start=(c == 0), stop=(c == KC - 1),
                        )
                rope_1024(kps[0:HD, :], kd2[half][0:HD, :], hsl, HD)
                nc.vector.tensor_copy(kd2[half][HD:P, :], kd2[half][0:HD, :])

            # ---- v projection: (s, d) via per-s-chunk matmuls ----
            for half in range(2):
                vps = ps_sc.tile([P, 1024], f32, tag="sc", name=f"vps{half}")
                for s in range(half * 8, half * 8 + 8):
                    idx = s % 8
                    vsl = slice(idx * HD, (idx + 1) * HD)
                    for c in range(KC):
                        nc.tensor.matmul(
                            vps[:, vsl],
                            lhsT=x_sb[c][:, s * P : (s + 1) * P],
                            rhs=kvw_sb[:, c, HD:P],
                            start=(c == 0), stop=(c == KC - 1),
                        )
                for s in range(half * 8, half * 8 + 8):
                    idx = s % 8
                    nc.scalar.copy(v_sb4[s // 4][:, s % 4, 0:HD],
                                   vps[:, idx * HD : (idx + 1) * HD])


            # ---- attention: units (hp, tq); 16 slices (s) each ----
            # slice scores [128, 1024] = {head 2hp q-block tq | head 2hp+1 same}
            # Emission per slice i: exp(i), QK(i+2), PV(i) -- the PE streams two
            # slices ahead, and with 3 score buffers the exp engines decouple.
            scale = 1.0 / float(np.sqrt(HD))
            units = [(hp, tq) for hp in range(2) for tq in range(4)]
            slices = [(u, s) for u in range(8) for s in range(SC)]
            ot_tiles = {}

            def emit_qk(i):
                u, s = slices[i]
                hp, tq = units[u]
                kd = kd2[s // 8]
                ssl = slice((s % 8) * P, (s % 8 + 1) * P)
                qr = qro4[hp][tq // 2]
                src = slice((tq % 2) * 512, (tq % 2) * 512 + 512)
                cur = ps_sc.tile([P, 1024], f32, tag="sc")
                nc.tensor.matmul(cur[:, 0:512], lhsT=kd[0:64, ssl],
                                 rhs=qr[0:64, src], start=True, stop=True)
                nc.tensor.matmul(cur[:, 512:1024], lhsT=kd[64:P, ssl],
                                 rhs=qr[64:P, src], start=True, stop=True)
                return cur

            def emit_norm(u, otA, otB):
                hp, tq = units[u]
                to = tq * 512
                # copy O^T+denom out of PSUM (frees ot tags for next unit);
                # otA via ACT, otB via DVE to split the copy cost
                for half, ot in ((0, otA), (1, otB)):
                    rows = slice(64 * half, 64 * half + 64)
                    of = norm.tile([HD + 1, 512], f32, tag=f"of{half}",
                                   name=f"of{half}")
                    nc.scalar.copy(of, ot[: HD + 1, :])
                    # custom-DVE recip and partition_broadcast both misbehave
                    # on base-partition-64 inputs; hop the denom row to
                    # partition 0 with a tiny sbuf->sbuf DMA first
                    dn = norm.tile([1, 512], f32, tag="dn")
                    nc.sync.dma_start(out=dn, in_=of[HD : HD + 1, :])
                    recip = norm.tile([1, 512], f32, tag="recip")
                    nc.vector.reciprocal_approx_fast(recip, dn)
                    rb = norm.tile([HD, 512], f32, tag="rb")
                    nc.gpsimd.partition_broadcast(rb, recip)
                    nc.vector.tensor_mul(
                        ot_sb[rows, hp, to : to + 512], of[0:HD, :], rb
                    )

            qk_bufs = {0: emit_qk(0), 1: emit_qk(1), 2: emit_qk(2)}
            for i, (u, s) in enumerate(slices):
                # q chunk 1 (heads 2-3) is projected inside the hp0 units --
                # it is first needed at slice 64, so these small matmul lumps
                # hide in the attention stream instead of lengthening the head
                if i == 8:
                    qproj_half(1, 0)
                elif i == 24:
                    qproj_half(1, 1)
                cur = qk_bufs.pop(i)
                if s == 0:
                    ot_tiles[u] = (ps_ot.tile([HD + 1, 512], f32, tag="otA",
                                              name=f"uotA{u}"),
                                   ps_ot.tile([HD + 1, 512], f32, tag="otB",
                                              name=f"uotB{u}"))
                otA, otB = ot_tiles[u]
                # exp: 10 of every 16 slices on ACT, 6 on DVE, interleaved
                if (i * 10) % 16 < 10:
                    pt = pexp.tile([P, 1024], bf16, tag="p")
                    nc.scalar.activation(pt, cur, AF.Exp, scale=scale)
                    pA, pB = pt[:, 0:512], pt[:, 512:1024]
                else:
                    st = psch.tile([P, 1024], i16, tag="q")
                    nc.vector.tensor_scalar(
                        out=st[:, :], in0=cur[:, :],
                        scalar1=SCH_A, scalar2=SCH_B,
                        op0=ALU.mult, op1=ALU.add,
                    )
                    pA = st[:, 0:512].bitcast(bf16)
                    pB = st[:, 512:1024].bitcast(bf16)
                if i + 3 < len(slices):
                    qk_bufs[i + 3] = emit_qk(i + 3)
                vst = v_sb4[s // 4][:, s % 4, 0 : HD + 1]
                nc.tensor.matmul(otA[:, :], lhsT=vst, rhs=pA,
                                 start=(s == 0), stop=(s == SC - 1))
                nc.tensor.matmul(otB[:, :], lhsT=vst, rhs=pB,
                                 start=(s == 0), stop=(s == SC - 1))
                if s == SC - 1:
                    emit_norm(u, otA, otB)

            if debug:
                for h in range(2):
                    nc.sync.dma_start(out=dbg_kdup_h[:, h * 1024 : (h + 1) * 1024],
                                      in_=kd2[h])
                for j in range(4):
                    nc.sync.dma_start(
                        out=dbg_v_h[:, j * 288 : (j + 1) * 288],
                        in_=v_sb4[j][:, :, :].rearrange("p a b -> p (a b)"),
                    )
                for m in range(2):
                    for h in range(2):
                        nc.sync.dma_start(
                            out=dbg_qro_h[:, (m * 2 + h) * 1024 : (m * 2 + h + 1) * 1024],
                            in_=qro4[m][h],
                        )
                nc.sync.dma_start(
                    out=dbg_ot_h[:, :],
                    in_=ot_sb[:, :, :].rearrange("p a b -> p (a b)"),
                )

            # ---- output projection: ps_sc ring provides the psum tiles ----
            for oc in range(KC):
                osl = slice(oc * P, (oc + 1) * P)
                halves = [ps_sc.tile([P, 1024], f32, tag="sc", name=f"ops{oc}a"),
                          ps_sc.tile([P, 1024], f32, tag="sc", name=f"ops{oc}b")]
                for th in range(2):
                    for t in range(2):
                        psl = slice(t * 512, (t + 1) * 512)
                        tsl = slice(th * 1024 + t * 512, th * 1024 + (t + 1) * 512)
                        for c in range(2):
                            nc.tensor.matmul(
                                halves[th][:, psl],
                                lhsT=owT_sb[:, c, osl],
                                rhs=ot_sb[:, c, tsl],
                                start=(c == 0),
                                stop=(c == 1),
                            )
                o_sb = outs.tile([P, T], bf16, tag="o")
                for th in range(2):
                    dst = o_sb[:, th * 1024 : (th + 1) * 1024]
                    if (oc + th) % 2 == 0:
                        nc.vector.tensor_copy(dst, halves[th])
                    else:
                        nc.scalar.copy(dst, halves[th])
                nc.sync.dma_start(out=outr[:, oc, :], in_=o_sb)

    nc.finalize()
    return nc


def _get_module():
    if "nc" not in _MODULE_CACHE:
        _MODULE_CACHE["nc"] = _build()
    return _MODULE_CACHE["nc"]


def _host_freqs(freqs_cos, freqs_sin):
    cos = np.asarray(freqs_cos, dtype=np.float32)  # (T, 32)
    sin = np.asarray(freqs_sin, dtype=np.float32)
    c64 = np.repeat(cos, 2, axis=1)                # (T, 64): col d -> cos[t, d//2]
    s64 = np.empty((T, HD), dtype=np.float32)
    s64[:, 0::2] = -sin
    s64[:, 1::2] = sin
    cosF = np.ascontiguousarray(np.concatenate([c64, c64], axis=1).T)  # (128, T)
    sinF = np.ascontiguousarray(np.concatenate([s64, s64], axis=1).T)
    return cosF, sinF


def kernel(x, q_w, kv_w, out_w, freqs_cos, freqs_sin):
    global LAST_RESULT
    import ml_dtypes
    from concourse.bass_utils import run_bass_kernel_spmd

    bf = ml_dtypes.bfloat16
    x = np.asarray(x, dtype=np.float32)
    q_w = np.asarray(q_w, dtype=np.float32)
    kv_w = np.asarray(kv_w, dtype=np.float32)
    out_w = np.asarray(out_w, dtype=np.float32)
    cosF, sinF = _host_freqs(freqs_cos, freqs_sin)

    xT = [np.ascontiguousarray(x[b].T).astype(bf) for b in range(B)]
    in_maps = []
    for core in range(8):
        b, g = core // KV, core % KV
        kvwT = np.concatenate(
            [kv_w[g * HD : (g + 1) * HD, :].T,
             kv_w[(KV + g) * HD : (KV + g + 1) * HD, :].T], axis=1
        )  # (D, 128): [kT | vT]
        in_maps.append(
            dict(
                xT=xT[b],
                qwT=np.ascontiguousarray(q_w[g * GH : (g + 1) * GH, :].T).astype(bf),
                kvwT=np.ascontiguousarray(kvwT).astype(bf),
                owT=np.ascontiguousarray(out_w[:, g * GH : (g + 1) * GH].T).astype(bf),
                cosF=cosF,
                sinF=sinF,
            )
        )

    nc = _get_module()
    trace = os.environ.get("KERNEL_TRACE", "0") == "1"
    res = run_bass_kernel_spmd(nc, in_maps, core_ids=list(range(8)), trace=trace)
    LAST_RESULT = res

    out = np.zeros((B, T, D), dtype=np.float32)
    for core in range(8):
        b = core // KV
        out[b] += res.results[core]["outT"].T.astype(np.float32)
    return out
